# revision 51
# baseline (speedup 1.0000x reference)
"""Masked attention (B=16, S=1024, H=1024) on 8 TRN2 NeuronCores.

Strategy: pure data-parallel over batch — 2 batches per core, no collectives.

Sparsity: the mask zeroes ~half of the key columns per batch; masked columns
give exactly-zero attention weights (exp(-1e9 - max) underflows in f32).  The
host permutes each batch's tokens so unmasked columns form a prefix, the
kernel runs attention over a compact key axis of UP columns, and the host
scatters the compact weights back into the dense [S, S] output.  Batches are
assigned to (core, slot) by descending unmasked count, so slot 0 carries the
wide batches and slot 1 compiles with a smaller UP (fewer key tiles).

Per batch (X = input[b] [S, H], XU = unmasked prefix [UP, H]):
  W2  = M @ XU^T                     -> [H, UP]   (M = Wq^T Wk / sqrt(H))
  s   = X @ W2 + d(row) + mkb(col)   -> [S, UP]
  e   = exp(s - rowmax)              (raw e + row sums out; host normalizes)
  vU  = XU @ Wv^T + bv               -> [UP, H]
  att = (e^T^T @ vU) / rowsum        -> e transposed on the PE; the recip
                                        rides the PSUM->SBUF epilogue copy

Computing W2 = M @ XU^T before the S-side contraction (instead of T1 = X @ M)
saves (S - UP)·H·H MACs per batch since UP < S.  All TensorEngine operands
bf16 (pre-cast on host), accumulation f32 in PSUM, softmax statistics f32.
"""
import numpy as np
import ml_dtypes

import concourse.bass as bass
import concourse.mybir as mybir
from concourse import bacc
from concourse.tile import TileContext
from concourse.bass_utils import run_bass_kernel_spmd
from concourse.masks import make_identity

B, S, H = 16, 1024, 1024
P = 128
NCORES = 8
B_LOC = B // NCORES          # batches per core
KT = H // P                  # 8 contraction tiles
RT = S // P                  # 8 query row blocks
NFREE = 512                  # matmul moving free dim (one PSUM bank)
BF16 = mybir.dt.bfloat16
F32 = mybir.dt.float32

_BUILD_CACHE = {}


def _chunks(total, step=NFREE):
    out = []
    o = 0
    while o < total:
        out.append((o, min(step, total - o)))
        o += step
    return out


def _uch(UP):
    if UP <= NFREE:
        return [(0, UP)]
    n_uch = (UP + NFREE - 1) // NFREE
    step = ((UP // n_uch) + 15) // 16 * 16
    out = []
    o = 0
    for i in range(n_uch):
        csz = step if i < n_uch - 1 else UP - o
        out.append((o, csz))
        o += csz
    return out


def build(UPS):
    """Build the SPMD graph; UPS[b] = compact key width for batch slot b."""
    UPS = tuple(UPS)
    if UPS in _BUILD_CACHE:
        return _BUILD_CACHE[UPS]
    UP0 = UPS[0]                       # widest slot (attwc/mkb allocation)
    assert UP0 == max(UPS) and all(u % 16 == 0 for u in UPS)
    NUT0 = (UP0 + P - 1) // P          # pool tiles sized for the widest slot
    HCH = _chunks(H)

    nc = bacc.Bacc()

    # All inputs arrive pre-tiled in SBUF layout (partition-major, contiguous
    # per partition) so DMA bursts are kilobytes, not 256B strided runs.
    xT = nc.declare_dram_parameter("xT", [B_LOC, P, KT, S], BF16, isOutput=False)
    mTp = nc.declare_dram_parameter("mTp", [KT, P, KT, P], BF16, isOutput=False)
    wvp = nc.declare_dram_parameter("wvp", [len(HCH), P, KT, NFREE], BF16, isOutput=False)
    mkb = nc.declare_dram_parameter("mkb", [B_LOC, P, UP0], BF16, isOutput=False)
    att = nc.declare_dram_parameter("att", [B_LOC, S, H], BF16, isOutput=True)
    attwc = nc.declare_dram_parameter("attwc", [B_LOC, S, UP0], BF16, isOutput=True)
    rs = nc.declare_dram_parameter("rs", [B_LOC, P, RT], F32, isOutput=True)

    with TileContext(nc) as tc:
        with (
            tc.tile_pool(name="const", bufs=1) as constp,
            tc.tile_pool(name="wpool", bufs=1) as wpool,
            tc.tile_pool(name="xpool", bufs=2) as xpool,
            tc.tile_pool(name="qkv", bufs=1) as qkvp,
            tc.tile_pool(name="soft", bufs=3) as soft,
            tc.tile_pool(name="stats", bufs=4) as stats,
            tc.tile_pool(name="psmm", bufs=6, space="PSUM") as psmm,
            tc.tile_pool(name="pstr", bufs=2, space="PSUM") as pstr,
        ):
            ident = constp.tile([P, P], BF16)
            make_identity(nc, ident)

            mT_t = wpool.tile([P, KT, KT, P], BF16)     # [p, h-block, h'-tile, h]
            wv_t = wpool.tile([P, len(HCH), KT, NFREE], BF16)
            xT0_t = xpool.tile([P, KT, S], BF16, name="xT0_t", tag="xT")

            # DMA issue order = first-use order.  The first W2 accumulation
            # chain needs mT[ot=0] plus the kt-ascending xTU column slices,
            # so those go first, split across the scalar and sync HWDGE
            # streams to halve issue latency.
            UH = _uch(UPS[0])[0][1]            # first u-chunk boundary
            nc.scalar.dma_start(out=mT_t[:, 0], in_=mTp[0])
            for j, kt2 in enumerate(range(0, KT, 2)):  # first-u-half, 2 kt each
                eng = nc.sync if j % 2 == 0 else nc.gpsimd
                eng.dma_start(out=xT0_t[:, kt2:kt2 + 2, 0:UH],
                              in_=xT[0][:, kt2:kt2 + 2, 0:UH])
            for ot in range(1, KT):
                nc.scalar.dma_start(out=mT_t[:, ot], in_=mTp[ot])
            if UH < UPS[0]:                    # second u-half feeds W2 uch1
                nc.gpsimd.dma_start(out=xT0_t[:, :, UH:UPS[0]],
                                    in_=xT[0][:, :, UH:UPS[0]])
            # V weights next; the query-side columns (only needed once scores
            # start, ~8us later) go last so they don't steal HBM bandwidth
            # from the W2 inputs during the critical head window.
            nc.scalar.dma_start(out=wv_t[:, 0], in_=wvp[0])
            if UPS[0] < S:
                nc.sync.dma_start(out=xT0_t[:, :, UPS[0]:S], in_=xT[0][:, :, UPS[0]:S])
            nc.scalar.dma_start(out=wv_t[:, 1], in_=wvp[1])

            def load_batch_small(b):
                UPb = UPS[b]
                mkb_t = xpool.tile([P, UP0], BF16, name="mkb_t", tag="mkb")
                nc.sync.dma_start(out=mkb_t[:, 0:UPb], in_=mkb[b][:, 0:UPb])
                return (mkb_t,)

            def load_batch_x(b):
                # issued on the scalar queue: it is busy with mT/wv until
                # ~15us, which keeps this 2MB prefetch from stealing HBM
                # bandwidth during the head window (sync races ahead).
                UPb = UPS[b]
                xb_t = xpool.tile([P, KT, S], BF16, name="xT_t", tag="xT")
                nc.scalar.dma_start(out=xb_t[:, :, 0:UPb], in_=xT[b][:, :, 0:UPb])
                if UPb < S:
                    nc.scalar.dma_start(out=xb_t[:, :, UPb:S], in_=xT[b][:, :, UPb:S])
                return xb_t

            small0 = load_batch_small(0)
            next_inputs = (xT0_t,) + small0
            for b in range(B_LOC):
                xT_t, mkb_t = next_inputs
                UP = UPS[b]
                UCH = _uch(UP)
                NUT = (UP + P - 1) // P
                PNS = [min(P, UP - ut * P) for ut in range(NUT)]
                xTU_t = xT_t[:, :, 0:UP]

                # ---- W2 = M @ XU^T  -> [h part, u free] bf16 ----
                # u-chunk-outer so the first half starts once half of xTU
                # has landed (head-latency trim for batch 0).
                w2_t = qkvp.tile([P, KT, UP0], BF16, name="w2_t", tag="w2", bufs=2)
                for off, csz in UCH:
                    for ot in range(KT):
                        ps_w = psmm.tile([P, NFREE], F32, name="ps_w", tag="mm")[:, :csz]
                        for kt in range(KT):
                            nc.tensor.matmul(ps_w, mT_t[:, ot, kt],
                                             xTU_t[:, kt, off:off + csz],
                                             start=(kt == 0), stop=(kt == KT - 1))
                        nc.scalar.activation(out=w2_t[:, ot, off:off + csz], in_=ps_w,
                                             func=mybir.ActivationFunctionType.Copy)

                # ---- attention pieces ----
                def emit_scores(r):
                    sc_t = soft.tile([P, UP0], F32, name="sc_t", tag="sc")
                    for off, csz in UCH:
                        sl = slice(off, off + csz)
                        ps_s = psmm.tile([P, NFREE], F32, name="ps_s", tag="mm")[:, :csz]
                        for kt in range(KT):
                            nc.tensor.matmul(ps_s, xT_t[:, kt, r * P:(r + 1) * P],
                                             w2_t[:, kt, sl], start=(kt == 0), stop=(kt == KT - 1))
                        # the q-side row bias cancels in the softmax, so only
                        # the column bias (mask + bk-term) is added.
                        nc.vector.tensor_tensor(out=sc_t[:, sl], in0=ps_s,
                                                in1=mkb_t[:, sl], op=mybir.AluOpType.add)
                    return sc_t

                def emit_softmax(r, sc_t):
                    # un-normalized weights go out raw (host divides by the
                    # row sums) — saves a DVE pass per row block.
                    negmax = stats.tile([P, 1], F32, name="negmax", tag="negmax")
                    nc.vector.reduce_max(out=negmax, in_=sc_t[:, 0:UP], axis=mybir.AxisListType.X, negate=True)
                    e_t = soft.tile([P, UP0], BF16, name="e_t", tag="e")
                    nc.scalar.activation(out=e_t[:, 0:UP], in_=sc_t[:, 0:UP],
                                         func=mybir.ActivationFunctionType.Exp,
                                         bias=negmax, scale=1.0, accum_out=rs_t[:, r:r + 1])
                    recip = stats.tile([P, 1], F32, name="recip", tag="recip")
                    nc.vector.reciprocal(out=recip, in_=rs_t[:, r:r + 1])
                    nc.sync.dma_start(out=attwc[b, r * P:(r + 1) * P, 0:UP], in_=e_t[:, 0:UP])
                    return e_t, recip

                def emit_tr(e_t):
                    # transpose e on the PE, one block ahead of its pv: the
                    # ACT copies land well before pv consumes them.
                    eT_t = soft.tile([P, NUT0, P], BF16, name="eT_t", tag="pT", bufs=4)
                    for ut in range(NUT):
                        pn = PNS[ut]
                        ps_t = pstr.tile([P, P], BF16, name="ps_t", tag="tr")
                        nc.tensor.transpose(ps_t[0:pn, :], e_t[:, ut * P:ut * P + pn], ident)
                        nc.scalar.activation(out=eT_t[0:pn, ut], in_=ps_t[0:pn, :],
                                             func=mybir.ActivationFunctionType.Copy)
                    return eT_t

                def emit_pv(r, eT_t, recip):
                    # att[i, h] = sum_u e[i, u] v[u, h] / rowsum[i]; the recip
                    # rides the PSUM->SBUF copy, split across ACT and DVE so
                    # neither queue's backlog delays the pool's bank release.
                    at_t = soft.tile([P, H], BF16, name="at_t", tag="at")
                    for ci, (off, csz) in enumerate(HCH):
                        sl = slice(off, off + csz)
                        ps_a = psmm.tile([P, NFREE], F32, name="ps_a", tag="mm")[:, :csz]
                        for ut in range(NUT):
                            pn = PNS[ut]
                            nc.tensor.matmul(ps_a, eT_t[0:pn, ut], v_t[0:pn, ut, sl],
                                             start=(ut == 0), stop=(ut == NUT - 1))
                        if ci == 0:
                            nc.scalar.activation(out=at_t[:, sl], in_=ps_a,
                                                 func=mybir.ActivationFunctionType.Copy,
                                                 scale=recip)
                        else:
                            nc.vector.tensor_scalar_mul(at_t[:, sl], ps_a, recip)
                    nc.sync.dma_start(out=att[b, r * P:(r + 1) * P, :], in_=at_t)

                # per-batch row sums, DMA'd out once at the end of the batch
                rs_t = stats.tile([P, RT], F32, name="rs_t", tag="rs", bufs=2)

                # two score blocks emitted up front so exp/softmax overlaps V
                sc0 = emit_scores(0)
                p0 = emit_softmax(0, sc0)
                sc1 = emit_scores(1)
                p1 = emit_softmax(1, sc1)

                # ---- vU[u, o] = XU @ Wv^T + bv ----
                v_t = qkvp.tile([P, NUT0, H], BF16, name="v_t", tag="v", bufs=2)
                for ci, (off, csz) in enumerate(HCH):
                    sl = slice(off, off + csz)
                    for ut in range(NUT):
                        pn = PNS[ut]
                        ps_v = psmm.tile([P, NFREE], F32, name="ps_v", tag="mm")[0:pn, :csz]
                        for kt in range(KT):
                            nc.tensor.matmul(ps_v, xTU_t[:, kt, ut * P:ut * P + pn],
                                             wv_t[:, ci, kt, 0:csz], start=(kt == 0), stop=(kt == KT - 1))
                        # bv is added on the host (sum of weights is 1 per row)
                        nc.vector.tensor_copy(out=v_t[0:pn, ut, sl], in_=ps_v)

                # Prefetch next batch's inputs now, so their sync-queue DMAs
                # sit ahead of this batch's output DMAs in the engine stream.
                if b + 1 < B_LOC:
                    nxt_x = load_batch_x(b + 1)
                    next_inputs = (nxt_x,) + load_batch_small(b + 1)

                # ---- software-pipelined row blocks ----
                # per iteration: sc(r), sm(r), tr(r-1), pv(r-2) — transposes
                # run one block ahead of their pv, softmax two ahead.
                pts = {0: p0, 1: p1}
                trs = {0: emit_tr(p0[0])}
                for r in range(2, RT):
                    sc_r = emit_scores(r)
                    pts[r] = emit_softmax(r, sc_r)
                    trs[r - 1] = emit_tr(pts[r - 1][0])
                    emit_pv(r - 2, trs[r - 2], pts[r - 2][1])
                trs[RT - 1] = emit_tr(pts[RT - 1][0])
                emit_pv(RT - 2, trs[RT - 2], pts[RT - 2][1])
                emit_pv(RT - 1, trs[RT - 1], pts[RT - 1][1])
                nc.sync.dma_start(out=rs[b], in_=rs_t)

    nc.finalize()
    _BUILD_CACHE[UPS] = nc
    return nc


def _bf16(x):
    return np.ascontiguousarray(x.astype(ml_dtypes.bfloat16))


def _roundup16(n):
    return max(P, ((n + 15) // 16) * 16)


def kernel(input, mask, Wq, bq, Wk, bk, Wv, bv):
    input = np.asarray(input, dtype=np.float32)
    mask = np.asarray(mask)
    scale = np.float32(1.0 / np.sqrt(H))

    # Fused scores: M = Wq^T Wk / sqrt(H); the bq row term feeds the
    # column bias c (folded into mkb), the bk row term feeds d.
    Wq = np.asarray(Wq, dtype=np.float32)
    Wk = np.asarray(Wk, dtype=np.float32)
    bq = np.asarray(bq, dtype=np.float32)
    bk = np.asarray(bk, dtype=np.float32)
    M = (Wq.T @ Wk) * scale
    w1 = (bq * scale) @ Wk               # column term: c[u] = XU[u] . w1
    bv = np.asarray(bv, dtype=np.float32)
    # The q-side row bias (bq term along queries) shifts every score in a row
    # equally, so it cancels in the softmax and is dropped entirely; bv is
    # added to att on the host since the weights sum to 1 per row.
    # Pre-tile weights: per-output-block, partition-major [blk, p, t, inner].
    # mTp holds M^T tiles (stationary for W2 = M @ XU^T).
    mTp = np.ascontiguousarray(
        _bf16(M.T).reshape(KT, P, KT, P).transpose(2, 1, 0, 3))
    wvp = np.ascontiguousarray(
        _bf16(np.asarray(Wv).T).reshape(KT, P, H // NFREE, NFREE).transpose(2, 1, 0, 3))

    # Permute each batch's token axis so unmasked tokens form a prefix: the
    # compact key/value block is then a slice of the (permuted) xT tile and
    # needs no separate transfer.  Queries are order-independent; outputs are
    # un-permuted below.
    m = np.asarray(mask[:, 0, 0, :])                     # [B, S]
    idxs = [np.nonzero(m[b] != 0)[0] for b in range(B)]
    ucounts = [len(ix) for ix in idxs]
    sparse = min(ucounts) > 0 and max(ucounts) < S
    if sparse:
        perms = [np.concatenate([idxs[b], np.nonzero(m[b] == 0)[0]]) for b in range(B)]
        # Assign batches to (core, slot) by descending unmasked count: slot 0
        # takes the 8 widest, slot 1 compiles against a narrower UP.
        order = np.argsort(np.asarray(ucounts), kind="stable")[::-1]
        asg = [[int(order[sl * NCORES + c]) for sl in range(B_LOC)]
               for c in range(NCORES)]                   # asg[core][slot] = batch
        UPS = [_roundup16(max(ucounts[asg[c][sl]] for c in range(NCORES)))
               for sl in range(B_LOC)]
    else:
        idxs = [np.arange(S) for _ in range(B)]
        ucounts = [S] * B
        perms = [np.arange(S) for _ in range(B)]
        asg = [[c * B_LOC + sl for sl in range(B_LOC)] for c in range(NCORES)]
        UPS = [S] * B_LOC
    UP0 = UPS[0]

    in_maps = []
    for c in range(NCORES):
        gbs = asg[c]
        xb = np.stack([input[gb][perms[gb]] for gb in gbs])  # [B_LOC, S, H]
        xTf = _bf16(xb.transpose(0, 2, 1))               # [B_LOC, H, S]
        mkb = np.zeros((B_LOC, P, UP0), dtype=ml_dtypes.bfloat16)
        for sl in range(B_LOC):
            gb = gbs[sl]
            UPb = UPS[sl]
            cvec = (xb[sl, :UPb].astype(np.float32) @ w1).astype(np.float32)
            mb = np.where(m[gb][perms[gb]][:UPb] == 0, np.float32(-1e9),
                          np.float32(0.0)) + cvec
            mkb[sl, :, :UPb] = mb.astype(ml_dtypes.bfloat16)[None, :]
        xT_t = np.ascontiguousarray(
            xTf.reshape(B_LOC, KT, P, S).transpose(0, 2, 1, 3))
        in_maps.append({
            "xT": xT_t,
            "mTp": mTp, "wvp": wvp, "mkb": mkb,
        })

    nc = build(UPS)
    res = run_bass_kernel_spmd(nc, in_maps, core_ids=list(range(NCORES)))
    att = np.empty((B, S, H), dtype=np.float32)
    attw = np.zeros((B, S, S), dtype=np.float32)
    for c in range(NCORES):
        att_c = res.results[c]["att"]                    # [B_LOC, S, H] bf16, permuted rows
        awc = res.results[c]["attwc"]                    # [B_LOC, S, UP0] bf16 raw exp
        rsc = res.results[c]["rs"]                       # [B_LOC, P, RT] f32 row sums
        for sl in range(B_LOC):
            gb = asg[c][sl]
            att[gb][perms[gb]] = att_c[sl].astype(np.float32) + bv
            rows = np.asarray(rsc[sl]).transpose(1, 0).reshape(S)   # per-query sums
            tmp = np.zeros((S, S), dtype=np.float32)
            tmp[:, idxs[gb]] = (awc[sl][:, :ucounts[gb]].astype(np.float32)
                                / rows[:, None])
            attw[gb][perms[gb]] = tmp
    return att, attw


# revision 53
# speedup vs baseline: 1.1783x; 1.1783x over previous
"""Masked attention (B=16, S=1024, H=1024) on 8 TRN2 NeuronCores.

Strategy: pure data-parallel over batch — 2 batches per core, no collectives.

Sparsity: the mask zeroes ~half of the key columns per batch; masked columns
give exactly-zero attention weights (exp(-1e9 - max) underflows in f32).  The
host permutes each batch's tokens so unmasked columns form a prefix, the
kernel runs attention over a compact key axis of UP columns, and the host
scatters the compact weights back into the dense [S, S] output.  Batches are
assigned to (core, slot) by descending unmasked count, so slot 0 carries the
wide batches and slot 1 compiles with a smaller UP (fewer key tiles).

Per batch (X = input[b] [S, H], XU = unmasked prefix [UP, H]):
  W2  = M @ XU^T                     -> [H, UP]   (M = Wq^T Wk / sqrt(H))
  s   = X @ W2 + d(row) + mkb(col)   -> [S, UP]
  e   = exp(s - rowmax)              (raw e + row sums out; host normalizes)
  vU  = XU @ Wv^T + bv               -> [UP, H]
  att = (e^T^T @ vU) / rowsum        -> e transposed on the PE; the recip
                                        rides the PSUM->SBUF epilogue copy

Computing W2 = M @ XU^T before the S-side contraction (instead of T1 = X @ M)
saves (S - UP)·H·H MACs per batch since UP < S.  All TensorEngine operands
bf16 (pre-cast on host), accumulation f32 in PSUM, softmax statistics f32.
"""
import numpy as np
import ml_dtypes

import concourse.bass as bass
import concourse.mybir as mybir
from concourse import bacc
from concourse.tile import TileContext
from concourse.bass_utils import run_bass_kernel_spmd
from concourse.masks import make_identity

B, S, H = 16, 1024, 1024
P = 128
NCORES = 8
B_LOC = B // NCORES          # batches per core
KT = H // P                  # 8 contraction tiles
RT = S // P                  # 8 query row blocks
NFREE = 512                  # matmul moving free dim (one PSUM bank)
BF16 = mybir.dt.bfloat16
F32 = mybir.dt.float32

_BUILD_CACHE = {}


def _chunks(total, step=NFREE):
    out = []
    o = 0
    while o < total:
        out.append((o, min(step, total - o)))
        o += step
    return out


def _uch(UP):
    if UP <= NFREE:
        return [(0, UP)]
    n_uch = (UP + NFREE - 1) // NFREE
    step = ((UP // n_uch) + 15) // 16 * 16
    out = []
    o = 0
    for i in range(n_uch):
        csz = step if i < n_uch - 1 else UP - o
        out.append((o, csz))
        o += csz
    return out


def build(UPS):
    """Build the SPMD graph; UPS[b] = compact key width for batch slot b."""
    UPS = tuple(UPS)
    if UPS in _BUILD_CACHE:
        return _BUILD_CACHE[UPS]
    UP0 = UPS[0]                       # widest slot (attwc/mkb allocation)
    assert UP0 == max(UPS) and all(u % 16 == 0 for u in UPS)
    NUT0 = (UP0 + P - 1) // P          # pool tiles sized for the widest slot
    HCH = _chunks(H)

    nc = bacc.Bacc()

    # All inputs arrive pre-tiled in SBUF layout (partition-major, contiguous
    # per partition) so DMA bursts are kilobytes, not 256B strided runs.
    xT = nc.declare_dram_parameter("xT", [B_LOC, P, KT, S], BF16, isOutput=False)
    mTp = nc.declare_dram_parameter("mTp", [KT, P, KT, P], BF16, isOutput=False)
    wvp = nc.declare_dram_parameter("wvp", [len(HCH), P, KT, NFREE], BF16, isOutput=False)
    mkb = nc.declare_dram_parameter("mkb", [B_LOC, P, UP0], BF16, isOutput=False)
    att = nc.declare_dram_parameter("att", [B_LOC, S, H], BF16, isOutput=True)
    attwc = nc.declare_dram_parameter("attwc", [B_LOC, S, UP0], BF16, isOutput=True)
    rs = nc.declare_dram_parameter("rs", [B_LOC, P, RT], F32, isOutput=True)

    with TileContext(nc) as tc:
        with (
            tc.tile_pool(name="const", bufs=1) as constp,
            tc.tile_pool(name="wpool", bufs=1) as wpool,
            tc.tile_pool(name="xpool", bufs=2) as xpool,
            tc.tile_pool(name="qkv", bufs=1) as qkvp,
            tc.tile_pool(name="soft", bufs=3) as soft,
            tc.tile_pool(name="stats", bufs=4) as stats,
            tc.tile_pool(name="psmm", bufs=6, space="PSUM") as psmm,
            tc.tile_pool(name="pstr", bufs=2, space="PSUM") as pstr,
        ):
            ident = constp.tile([P, P], BF16)
            make_identity(nc, ident)

            mT_t = wpool.tile([P, KT, KT, P], BF16)     # [p, h-block, h'-tile, h]
            wv_t = wpool.tile([P, len(HCH), KT, NFREE], BF16)
            xT0_t = xpool.tile([P, KT, S], BF16, name="xT0_t", tag="xT")

            # DMA issue order = first-use order.  The first W2 accumulation
            # chain needs mT[ot=0] plus the kt-ascending xTU column slices,
            # so those go first, split across the scalar and sync HWDGE
            # streams to halve issue latency.
            UH = _uch(UPS[0])[0][1]            # first u-chunk boundary
            nc.scalar.dma_start(out=mT_t[:, 0], in_=mTp[0])
            for j, kt2 in enumerate(range(0, KT, 2)):  # first-u-half, 2 kt each
                eng = nc.sync if j % 2 == 0 else nc.gpsimd
                eng.dma_start(out=xT0_t[:, kt2:kt2 + 2, 0:UH],
                              in_=xT[0][:, kt2:kt2 + 2, 0:UH])
            for ot in range(1, KT):
                nc.scalar.dma_start(out=mT_t[:, ot], in_=mTp[ot])
            if UH < UPS[0]:                    # second u-half feeds W2 uch1
                nc.gpsimd.dma_start(out=xT0_t[:, :, UH:UPS[0]],
                                    in_=xT[0][:, :, UH:UPS[0]])
            # V weights next; the query-tail columns stay on sync — a third
            # writer queue (scalar) on xT0_t raced intermittently (1-in-3
            # wrong results), so only sync+gpsimd ever write this tile.
            nc.scalar.dma_start(out=wv_t[:, 0], in_=wvp[0])
            if UPS[0] < S:
                nc.sync.dma_start(out=xT0_t[:, :, UPS[0]:S], in_=xT[0][:, :, UPS[0]:S])
            nc.scalar.dma_start(out=wv_t[:, 1], in_=wvp[1])

            def load_batch_small(b):
                UPb = UPS[b]
                mkb_t = xpool.tile([P, UP0], BF16, name="mkb_t", tag="mkb")
                nc.sync.dma_start(out=mkb_t[:, 0:UPb], in_=mkb[b][:, 0:UPb])
                return (mkb_t,)

            def load_batch_x(b):
                # issued on the scalar queue: it is busy with mT/wv until
                # ~15us, which keeps this 2MB prefetch from stealing HBM
                # bandwidth during the head window (sync races ahead).
                UPb = UPS[b]
                xb_t = xpool.tile([P, KT, S], BF16, name="xT_t", tag="xT")
                nc.scalar.dma_start(out=xb_t[:, :, 0:UPb], in_=xT[b][:, :, 0:UPb])
                if UPb < S:
                    nc.scalar.dma_start(out=xb_t[:, :, UPb:S], in_=xT[b][:, :, UPb:S])
                return xb_t

            small0 = load_batch_small(0)
            next_inputs = (xT0_t,) + small0
            for b in range(B_LOC):
                xT_t, mkb_t = next_inputs
                UP = UPS[b]
                UCH = _uch(UP)
                NUT = (UP + P - 1) // P
                PNS = [min(P, UP - ut * P) for ut in range(NUT)]
                xTU_t = xT_t[:, :, 0:UP]

                # ---- W2 = M @ XU^T  -> [h part, u free] bf16 ----
                # u-chunk-outer so the first half starts once half of xTU
                # has landed (head-latency trim for batch 0).
                w2_t = qkvp.tile([P, KT, UP0], BF16, name="w2_t", tag="w2", bufs=2)
                for off, csz in UCH:
                    for ot in range(KT):
                        ps_w = psmm.tile([P, NFREE], F32, name="ps_w", tag="mm")[:, :csz]
                        for kt in range(KT):
                            nc.tensor.matmul(ps_w, mT_t[:, ot, kt],
                                             xTU_t[:, kt, off:off + csz],
                                             start=(kt == 0), stop=(kt == KT - 1))
                        nc.scalar.activation(out=w2_t[:, ot, off:off + csz], in_=ps_w,
                                             func=mybir.ActivationFunctionType.Copy)

                # ---- attention pieces ----
                def emit_scores(r):
                    sc_t = soft.tile([P, UP0], F32, name="sc_t", tag="sc")
                    for off, csz in UCH:
                        sl = slice(off, off + csz)
                        ps_s = psmm.tile([P, NFREE], F32, name="ps_s", tag="mm")[:, :csz]
                        for kt in range(KT):
                            nc.tensor.matmul(ps_s, xT_t[:, kt, r * P:(r + 1) * P],
                                             w2_t[:, kt, sl], start=(kt == 0), stop=(kt == KT - 1))
                        # the q-side row bias cancels in the softmax, so only
                        # the column bias (mask + bk-term) is added.
                        nc.vector.tensor_tensor(out=sc_t[:, sl], in0=ps_s,
                                                in1=mkb_t[:, sl], op=mybir.AluOpType.add)
                    return sc_t

                def emit_softmax(r, sc_t):
                    # un-normalized weights go out raw (host divides by the
                    # row sums) — saves a DVE pass per row block.
                    negmax = stats.tile([P, 1], F32, name="negmax", tag="negmax")
                    nc.vector.reduce_max(out=negmax, in_=sc_t[:, 0:UP], axis=mybir.AxisListType.X, negate=True)
                    e_t = soft.tile([P, UP0], BF16, name="e_t", tag="e")
                    nc.scalar.activation(out=e_t[:, 0:UP], in_=sc_t[:, 0:UP],
                                         func=mybir.ActivationFunctionType.Exp,
                                         bias=negmax, scale=1.0, accum_out=rs_t[:, r:r + 1])
                    recip = stats.tile([P, 1], F32, name="recip", tag="recip")
                    nc.vector.reciprocal(out=recip, in_=rs_t[:, r:r + 1])
                    nc.sync.dma_start(out=attwc[b, r * P:(r + 1) * P, 0:UP], in_=e_t[:, 0:UP])
                    return e_t, recip

                def emit_tr(e_t):
                    # transpose e on the PE, one block ahead of its pv: the
                    # ACT copies land well before pv consumes them.
                    eT_t = soft.tile([P, NUT0, P], BF16, name="eT_t", tag="pT", bufs=4)
                    for ut in range(NUT):
                        pn = PNS[ut]
                        ps_t = pstr.tile([P, P], BF16, name="ps_t", tag="tr")
                        nc.tensor.transpose(ps_t[0:pn, :], e_t[:, ut * P:ut * P + pn], ident)
                        nc.scalar.activation(out=eT_t[0:pn, ut], in_=ps_t[0:pn, :],
                                             func=mybir.ActivationFunctionType.Copy)
                    return eT_t

                def emit_pv(r, eT_t, recip):
                    # att[i, h] = sum_u e[i, u] v[u, h] / rowsum[i]; the recip
                    # rides the PSUM->SBUF copy, split across ACT and DVE so
                    # neither queue's backlog delays the pool's bank release.
                    at_t = soft.tile([P, H], BF16, name="at_t", tag="at")
                    for ci, (off, csz) in enumerate(HCH):
                        sl = slice(off, off + csz)
                        ps_a = psmm.tile([P, NFREE], F32, name="ps_a", tag="mm")[:, :csz]
                        for ut in range(NUT):
                            pn = PNS[ut]
                            nc.tensor.matmul(ps_a, eT_t[0:pn, ut], v_t[0:pn, ut, sl],
                                             start=(ut == 0), stop=(ut == NUT - 1))
                        if ci == 0:
                            nc.scalar.activation(out=at_t[:, sl], in_=ps_a,
                                                 func=mybir.ActivationFunctionType.Copy,
                                                 scale=recip)
                        else:
                            nc.vector.tensor_scalar_mul(at_t[:, sl], ps_a, recip)
                    nc.sync.dma_start(out=att[b, r * P:(r + 1) * P, :], in_=at_t)

                # per-batch row sums, DMA'd out once at the end of the batch
                rs_t = stats.tile([P, RT], F32, name="rs_t", tag="rs", bufs=2)

                # two score blocks emitted up front so exp/softmax overlaps V
                sc0 = emit_scores(0)
                p0 = emit_softmax(0, sc0)
                sc1 = emit_scores(1)
                p1 = emit_softmax(1, sc1)

                # ---- vU[u, o] = XU @ Wv^T + bv ----
                v_t = qkvp.tile([P, NUT0, H], BF16, name="v_t", tag="v", bufs=2)
                for ci, (off, csz) in enumerate(HCH):
                    sl = slice(off, off + csz)
                    for ut in range(NUT):
                        pn = PNS[ut]
                        ps_v = psmm.tile([P, NFREE], F32, name="ps_v", tag="mm")[0:pn, :csz]
                        for kt in range(KT):
                            nc.tensor.matmul(ps_v, xTU_t[:, kt, ut * P:ut * P + pn],
                                             wv_t[:, ci, kt, 0:csz], start=(kt == 0), stop=(kt == KT - 1))
                        # bv is added on the host (sum of weights is 1 per row)
                        nc.vector.tensor_copy(out=v_t[0:pn, ut, sl], in_=ps_v)

                # Prefetch next batch's inputs now, so their sync-queue DMAs
                # sit ahead of this batch's output DMAs in the engine stream.
                if b + 1 < B_LOC:
                    nxt_x = load_batch_x(b + 1)
                    next_inputs = (nxt_x,) + load_batch_small(b + 1)

                # ---- software-pipelined row blocks ----
                # per iteration: sc(r), sm(r), tr(r-1), pv(r-2) — transposes
                # run one block ahead of their pv, softmax two ahead.
                pts = {0: p0, 1: p1}
                trs = {0: emit_tr(p0[0])}
                for r in range(2, RT):
                    sc_r = emit_scores(r)
                    pts[r] = emit_softmax(r, sc_r)
                    trs[r - 1] = emit_tr(pts[r - 1][0])
                    emit_pv(r - 2, trs[r - 2], pts[r - 2][1])
                trs[RT - 1] = emit_tr(pts[RT - 1][0])
                emit_pv(RT - 2, trs[RT - 2], pts[RT - 2][1])
                emit_pv(RT - 1, trs[RT - 1], pts[RT - 1][1])
                nc.sync.dma_start(out=rs[b], in_=rs_t)

    nc.finalize()
    _BUILD_CACHE[UPS] = nc
    return nc


def _bf16(x):
    return np.ascontiguousarray(x.astype(ml_dtypes.bfloat16))


def _roundup16(n):
    return max(P, ((n + 15) // 16) * 16)


def kernel(input, mask, Wq, bq, Wk, bk, Wv, bv):
    input = np.asarray(input, dtype=np.float32)
    mask = np.asarray(mask)
    scale = np.float32(1.0 / np.sqrt(H))

    # Fused scores: M = Wq^T Wk / sqrt(H); the bq row term feeds the
    # column bias c (folded into mkb), the bk row term feeds d.
    Wq = np.asarray(Wq, dtype=np.float32)
    Wk = np.asarray(Wk, dtype=np.float32)
    bq = np.asarray(bq, dtype=np.float32)
    bk = np.asarray(bk, dtype=np.float32)
    M = (Wq.T @ Wk) * scale
    w1 = (bq * scale) @ Wk               # column term: c[u] = XU[u] . w1
    bv = np.asarray(bv, dtype=np.float32)
    # The q-side row bias (bq term along queries) shifts every score in a row
    # equally, so it cancels in the softmax and is dropped entirely; bv is
    # added to att on the host since the weights sum to 1 per row.
    # Pre-tile weights: per-output-block, partition-major [blk, p, t, inner].
    # mTp holds M^T tiles (stationary for W2 = M @ XU^T).
    mTp = np.ascontiguousarray(
        _bf16(M.T).reshape(KT, P, KT, P).transpose(2, 1, 0, 3))
    wvp = np.ascontiguousarray(
        _bf16(np.asarray(Wv).T).reshape(KT, P, H // NFREE, NFREE).transpose(2, 1, 0, 3))

    # Permute each batch's token axis so unmasked tokens form a prefix: the
    # compact key/value block is then a slice of the (permuted) xT tile and
    # needs no separate transfer.  Queries are order-independent; outputs are
    # un-permuted below.
    m = np.asarray(mask[:, 0, 0, :])                     # [B, S]
    idxs = [np.nonzero(m[b] != 0)[0] for b in range(B)]
    ucounts = [len(ix) for ix in idxs]
    sparse = min(ucounts) > 0 and max(ucounts) < S
    if sparse:
        perms = [np.concatenate([idxs[b], np.nonzero(m[b] == 0)[0]]) for b in range(B)]
        # Assign batches to (core, slot) by descending unmasked count: slot 0
        # takes the 8 widest, slot 1 compiles against a narrower UP.
        order = np.argsort(np.asarray(ucounts), kind="stable")[::-1]
        asg = [[int(order[sl * NCORES + c]) for sl in range(B_LOC)]
               for c in range(NCORES)]                   # asg[core][slot] = batch
        UPS = [_roundup16(max(ucounts[asg[c][sl]] for c in range(NCORES)))
               for sl in range(B_LOC)]
    else:
        idxs = [np.arange(S) for _ in range(B)]
        ucounts = [S] * B
        perms = [np.arange(S) for _ in range(B)]
        asg = [[c * B_LOC + sl for sl in range(B_LOC)] for c in range(NCORES)]
        UPS = [S] * B_LOC
    UP0 = UPS[0]

    in_maps = []
    for c in range(NCORES):
        gbs = asg[c]
        xb = np.stack([input[gb][perms[gb]] for gb in gbs])  # [B_LOC, S, H]
        xTf = _bf16(xb.transpose(0, 2, 1))               # [B_LOC, H, S]
        mkb = np.zeros((B_LOC, P, UP0), dtype=ml_dtypes.bfloat16)
        for sl in range(B_LOC):
            gb = gbs[sl]
            UPb = UPS[sl]
            cvec = (xb[sl, :UPb].astype(np.float32) @ w1).astype(np.float32)
            mb = np.where(m[gb][perms[gb]][:UPb] == 0, np.float32(-1e9),
                          np.float32(0.0)) + cvec
            mkb[sl, :, :UPb] = mb.astype(ml_dtypes.bfloat16)[None, :]
        xT_t = np.ascontiguousarray(
            xTf.reshape(B_LOC, KT, P, S).transpose(0, 2, 1, 3))
        in_maps.append({
            "xT": xT_t,
            "mTp": mTp, "wvp": wvp, "mkb": mkb,
        })

    nc = build(UPS)
    res = run_bass_kernel_spmd(nc, in_maps, core_ids=list(range(NCORES)))
    att = np.empty((B, S, H), dtype=np.float32)
    attw = np.zeros((B, S, S), dtype=np.float32)
    for c in range(NCORES):
        att_c = res.results[c]["att"]                    # [B_LOC, S, H] bf16, permuted rows
        awc = res.results[c]["attwc"]                    # [B_LOC, S, UP0] bf16 raw exp
        rsc = res.results[c]["rs"]                       # [B_LOC, P, RT] f32 row sums
        for sl in range(B_LOC):
            gb = asg[c][sl]
            att[gb][perms[gb]] = att_c[sl].astype(np.float32) + bv
            rows = np.asarray(rsc[sl]).transpose(1, 0).reshape(S)   # per-query sums
            tmp = np.zeros((S, S), dtype=np.float32)
            tmp[:, idxs[gb]] = (awc[sl][:, :ucounts[gb]].astype(np.float32)
                                / rows[:, None])
            attw[gb][perms[gb]] = tmp
    return att, attw


# revision 59
# speedup vs baseline: 1.2012x; 1.0194x over previous
"""Masked attention (B=16, S=1024, H=1024) on 8 TRN2 NeuronCores.

Strategy: pure data-parallel over batch — 2 batches per core, no collectives.

Sparsity: the mask zeroes ~half of the key columns per batch; masked columns
give exactly-zero attention weights (exp(-1e9 - max) underflows in f32).  The
host permutes each batch's tokens so unmasked columns form a prefix, the
kernel runs attention over a compact key axis of UP columns, and the host
scatters the compact weights back into the dense [S, S] output.  Batches are
assigned to (core, slot) by descending unmasked count, so slot 0 carries the
wide batches and slot 1 compiles with a smaller UP (fewer key tiles).

Per batch (X = input[b] [S, H], XU = unmasked prefix [UP, H]):
  W2  = M @ XU^T                     -> [H, UP]   (M = Wq^T Wk / sqrt(H))
  s   = X @ W2 + d(row) + mkb(col)   -> [S, UP]
  e   = exp(s - rowmax)              (raw e + row sums out; host normalizes)
  vU  = XU @ Wv^T + bv               -> [UP, H]
  att = (e^T^T @ vU) / rowsum        -> e transposed on the PE; the recip
                                        rides the PSUM->SBUF epilogue copy

Computing W2 = M @ XU^T before the S-side contraction (instead of T1 = X @ M)
saves (S - UP)·H·H MACs per batch since UP < S.  All TensorEngine operands
bf16 (pre-cast on host), accumulation f32 in PSUM, softmax statistics f32.
"""
import numpy as np
import ml_dtypes

import concourse.bass as bass
import concourse.mybir as mybir
from concourse import bacc
from concourse.tile import TileContext
from concourse.bass_utils import run_bass_kernel_spmd
from concourse.masks import make_identity

B, S, H = 16, 1024, 1024
P = 128
NCORES = 8
B_LOC = B // NCORES          # batches per core
KT = H // P                  # 8 contraction tiles
RT = S // P                  # 8 query row blocks
NFREE = 512                  # matmul moving free dim (one PSUM bank)
BF16 = mybir.dt.bfloat16
F32 = mybir.dt.float32

_BUILD_CACHE = {}


def _chunks(total, step=NFREE):
    out = []
    o = 0
    while o < total:
        out.append((o, min(step, total - o)))
        o += step
    return out


def _uch(UP):
    if UP <= NFREE:
        return [(0, UP)]
    n_uch = (UP + NFREE - 1) // NFREE
    step = ((UP // n_uch) + 15) // 16 * 16
    out = []
    o = 0
    for i in range(n_uch):
        csz = step if i < n_uch - 1 else UP - o
        out.append((o, csz))
        o += csz
    return out


def build(UPS):
    """Build the SPMD graph; UPS[b] = compact key width for batch slot b."""
    UPS = tuple(UPS)
    if UPS in _BUILD_CACHE:
        return _BUILD_CACHE[UPS]
    UP0 = UPS[0]                       # widest slot (attwc/mkb allocation)
    assert UP0 == max(UPS) and all(u % 16 == 0 for u in UPS)
    NUT0 = (UP0 + P - 1) // P          # pool tiles sized for the widest slot
    HCH = _chunks(H)

    nc = bacc.Bacc()

    # All inputs arrive pre-tiled in SBUF layout (partition-major, contiguous
    # per partition) so DMA bursts are kilobytes, not 256B strided runs.
    xT = nc.declare_dram_parameter("xT", [B_LOC, P, KT, S], BF16, isOutput=False)
    mTp = nc.declare_dram_parameter("mTp", [KT, P, KT, P], BF16, isOutput=False)
    wvp = nc.declare_dram_parameter("wvp", [len(HCH), P, KT, NFREE], BF16, isOutput=False)
    mkb = nc.declare_dram_parameter("mkb", [B_LOC, P, UP0], BF16, isOutput=False)
    att = nc.declare_dram_parameter("att", [B_LOC, S, H], BF16, isOutput=True)
    attwc = nc.declare_dram_parameter("attwc", [B_LOC, S, UP0], BF16, isOutput=True)
    rs = nc.declare_dram_parameter("rs", [B_LOC, P, RT], F32, isOutput=True)

    with TileContext(nc) as tc:
        with (
            tc.tile_pool(name="const", bufs=1) as constp,
            tc.tile_pool(name="wpool", bufs=1) as wpool,
            tc.tile_pool(name="xpool", bufs=2) as xpool,
            tc.tile_pool(name="qkv", bufs=1) as qkvp,
            tc.tile_pool(name="soft", bufs=3) as soft,
            tc.tile_pool(name="stats", bufs=4) as stats,
            tc.tile_pool(name="psmm", bufs=6, space="PSUM") as psmm,
            tc.tile_pool(name="pstr", bufs=2, space="PSUM") as pstr,
        ):
            ident = constp.tile([P, P], BF16)
            make_identity(nc, ident)

            mT_t = wpool.tile([P, KT, KT, P], BF16)     # [p, h-block, h'-tile, h]
            wv_t = wpool.tile([P, len(HCH), KT, NFREE], BF16)
            # The token axis is split at the 128-aligned key boundary XSP:
            # the main tile (keys + early query blocks) is written by
            # sync/gpsimd, the query tail by scalar ONLY — three writer
            # queues on one tile raced intermittently.
            XSP = min(S, NUT0 * P)
            xT0_t = xpool.tile([P, KT, XSP], BF16, name="xT0_t", tag="xT")
            xq0_t = (xpool.tile([P, KT, S - XSP], BF16, name="xq0_t", tag="xq")
                     if XSP < S else None)

            # DMA issue order = first-use order.  The first W2 accumulation
            # chain needs mT[ot=0] plus the kt-ascending xTU column slices,
            # so those go first, split across the scalar and sync HWDGE
            # streams to halve issue latency.
            UH = _uch(UPS[0])[0][1]            # first u-chunk boundary
            nc.scalar.dma_start(out=mT_t[:, 0], in_=mTp[0])
            for j, kt2 in enumerate(range(0, KT, 2)):  # first-u-half, 2 kt each
                eng = nc.sync if j % 2 == 0 else nc.gpsimd
                eng.dma_start(out=xT0_t[:, kt2:kt2 + 2, 0:UH],
                              in_=xT[0][:, kt2:kt2 + 2, 0:UH])
            for ot in range(1, KT):
                nc.scalar.dma_start(out=mT_t[:, ot], in_=mTp[ot])
            if UH < UPS[0]:                    # second u-half feeds W2 uch1
                nc.gpsimd.dma_start(out=xT0_t[:, :, UH:UPS[0]],
                                    in_=xT[0][:, :, UH:UPS[0]])
            if UPS[0] < XSP:                   # key-boundary pad queries
                nc.sync.dma_start(out=xT0_t[:, :, UPS[0]:XSP],
                                  in_=xT[0][:, :, UPS[0]:XSP])
            # V weights; the query tail rides the busy scalar queue after
            # them — it is only read by score blocks past XSP (~40us in),
            # and delaying it frees ~0.75MB of HBM bandwidth in the
            # starved head window.
            nc.scalar.dma_start(out=wv_t[:, 0], in_=wvp[0])
            nc.scalar.dma_start(out=wv_t[:, 1], in_=wvp[1])
            if XSP < S:
                nc.scalar.dma_start(out=xq0_t, in_=xT[0][:, :, XSP:S])

            def load_batch_small(b):
                UPb = UPS[b]
                mkb_t = xpool.tile([P, UP0], BF16, name="mkb_t", tag="mkb")
                nc.sync.dma_start(out=mkb_t[:, 0:UPb], in_=mkb[b][:, 0:UPb])
                return (mkb_t,)

            def load_batch_x(b):
                # issued on the scalar queue: it is busy with mT/wv until
                # ~15us, which keeps this 2MB prefetch from stealing HBM
                # bandwidth during the head window (sync races ahead).
                UPb = UPS[b]
                xb_t = xpool.tile([P, KT, XSP], BF16, name="xT_t", tag="xT")
                nc.scalar.dma_start(out=xb_t[:, :, 0:UPb], in_=xT[b][:, :, 0:UPb])
                if UPb < XSP:
                    nc.scalar.dma_start(out=xb_t[:, :, UPb:XSP], in_=xT[b][:, :, UPb:XSP])
                if XSP < S:
                    xqb_t = xpool.tile([P, KT, S - XSP], BF16, name="xq_t", tag="xq")
                    nc.scalar.dma_start(out=xqb_t, in_=xT[b][:, :, XSP:S])
                else:
                    xqb_t = None
                return xb_t, xqb_t

            small0 = load_batch_small(0)
            next_inputs = (xT0_t, xq0_t) + small0
            for b in range(B_LOC):
                xT_t, xq_t, mkb_t = next_inputs
                UP = UPS[b]
                UCH = _uch(UP)
                NUT = (UP + P - 1) // P
                PNS = [min(P, UP - ut * P) for ut in range(NUT)]
                xTU_t = xT_t[:, :, 0:UP]

                # ---- W2 = M @ XU^T  -> [h part, u free] bf16 ----
                # u-chunk-outer so the first half starts once half of xTU
                # has landed (head-latency trim for batch 0).
                w2_t = qkvp.tile([P, KT, UP0], BF16, name="w2_t", tag="w2", bufs=2)
                for off, csz in UCH:
                    for ot in range(KT):
                        ps_w = psmm.tile([P, NFREE], F32, name="ps_w", tag="mm")[:, :csz]
                        for kt in range(KT):
                            nc.tensor.matmul(ps_w, mT_t[:, ot, kt],
                                             xTU_t[:, kt, off:off + csz],
                                             start=(kt == 0), stop=(kt == KT - 1))
                        nc.scalar.activation(out=w2_t[:, ot, off:off + csz], in_=ps_w,
                                             func=mybir.ActivationFunctionType.Copy)

                # ---- attention pieces ----
                def emit_scores(r):
                    base = r * P
                    sc_t = soft.tile([P, UP0], F32, name="sc_t", tag="sc")
                    for off, csz in UCH:
                        sl = slice(off, off + csz)
                        ps_s = psmm.tile([P, NFREE], F32, name="ps_s", tag="mm")[:, :csz]
                        for kt in range(KT):
                            if base < XSP:
                                lhs = xT_t[:, kt, base:base + P]
                            else:
                                lhs = xq_t[:, kt, base - XSP:base - XSP + P]
                            nc.tensor.matmul(ps_s, lhs,
                                             w2_t[:, kt, sl], start=(kt == 0), stop=(kt == KT - 1))
                        # the q-side row bias cancels in the softmax, so only
                        # the column bias (mask + bk-term) is added.
                        nc.vector.tensor_tensor(out=sc_t[:, sl], in0=ps_s,
                                                in1=mkb_t[:, sl], op=mybir.AluOpType.add)
                    return sc_t

                def emit_softmax(r, sc_t):
                    # un-normalized weights go out raw (host divides by the
                    # row sums) — saves a DVE pass per row block.
                    negmax = stats.tile([P, 1], F32, name="negmax", tag="negmax")
                    nc.vector.reduce_max(out=negmax, in_=sc_t[:, 0:UP], axis=mybir.AxisListType.X, negate=True)
                    e_t = soft.tile([P, UP0], BF16, name="e_t", tag="e")
                    nc.scalar.activation(out=e_t[:, 0:UP], in_=sc_t[:, 0:UP],
                                         func=mybir.ActivationFunctionType.Exp,
                                         bias=negmax, scale=1.0, accum_out=rs_t[:, r:r + 1])
                    recip = stats.tile([P, 1], F32, name="recip", tag="recip")
                    nc.vector.reciprocal(out=recip, in_=rs_t[:, r:r + 1])
                    nc.sync.dma_start(out=attwc[b, r * P:(r + 1) * P, 0:UP], in_=e_t[:, 0:UP])
                    return e_t, recip

                def emit_tr(e_t):
                    # transpose e on the PE, one block ahead of its pv: the
                    # ACT copies land well before pv consumes them.
                    eT_t = soft.tile([P, NUT0, P], BF16, name="eT_t", tag="pT", bufs=4)
                    for ut in range(NUT):
                        pn = PNS[ut]
                        ps_t = pstr.tile([P, P], BF16, name="ps_t", tag="tr")
                        nc.tensor.transpose(ps_t[0:pn, :], e_t[:, ut * P:ut * P + pn], ident)
                        nc.scalar.activation(out=eT_t[0:pn, ut], in_=ps_t[0:pn, :],
                                             func=mybir.ActivationFunctionType.Copy)
                    return eT_t

                def emit_pv(r, eT_t, recip):
                    # att[i, h] = sum_u e[i, u] v[u, h] / rowsum[i]; the recip
                    # rides the PSUM->SBUF copy, split across ACT and DVE so
                    # neither queue's backlog delays the pool's bank release.
                    at_t = soft.tile([P, H], BF16, name="at_t", tag="at")
                    for ci, (off, csz) in enumerate(HCH):
                        sl = slice(off, off + csz)
                        ps_a = psmm.tile([P, NFREE], F32, name="ps_a", tag="mm")[:, :csz]
                        for ut in range(NUT):
                            pn = PNS[ut]
                            nc.tensor.matmul(ps_a, eT_t[0:pn, ut], v_t[0:pn, ut, sl],
                                             start=(ut == 0), stop=(ut == NUT - 1))
                        if ci == 0:
                            nc.scalar.activation(out=at_t[:, sl], in_=ps_a,
                                                 func=mybir.ActivationFunctionType.Copy,
                                                 scale=recip)
                        else:
                            nc.vector.tensor_scalar_mul(at_t[:, sl], ps_a, recip)
                    nc.sync.dma_start(out=att[b, r * P:(r + 1) * P, :], in_=at_t)

                # per-batch row sums, DMA'd out once at the end of the batch
                rs_t = stats.tile([P, RT], F32, name="rs_t", tag="rs", bufs=2)

                # two score blocks emitted up front so exp/softmax overlaps V
                sc0 = emit_scores(0)
                p0 = emit_softmax(0, sc0)
                sc1 = emit_scores(1)
                p1 = emit_softmax(1, sc1)

                # ---- vU[u, o] = XU @ Wv^T + bv ----
                v_t = qkvp.tile([P, NUT0, H], BF16, name="v_t", tag="v", bufs=2)
                for ci, (off, csz) in enumerate(HCH):
                    sl = slice(off, off + csz)
                    for ut in range(NUT):
                        pn = PNS[ut]
                        ps_v = psmm.tile([P, NFREE], F32, name="ps_v", tag="mm")[0:pn, :csz]
                        for kt in range(KT):
                            nc.tensor.matmul(ps_v, xTU_t[:, kt, ut * P:ut * P + pn],
                                             wv_t[:, ci, kt, 0:csz], start=(kt == 0), stop=(kt == KT - 1))
                        # bv is added on the host (sum of weights is 1 per row)
                        nc.vector.tensor_copy(out=v_t[0:pn, ut, sl], in_=ps_v)

                # Prefetch next batch's inputs now, so their sync-queue DMAs
                # sit ahead of this batch's output DMAs in the engine stream.
                if b + 1 < B_LOC:
                    nxt_x = load_batch_x(b + 1)
                    next_inputs = nxt_x + load_batch_small(b + 1)

                # ---- software-pipelined row blocks ----
                # per iteration: sc(r), sm(r), tr(r-1), pv(r-2) — transposes
                # run one block ahead of their pv, softmax two ahead.
                pts = {0: p0, 1: p1}
                trs = {0: emit_tr(p0[0])}
                for r in range(2, RT):
                    sc_r = emit_scores(r)
                    pts[r] = emit_softmax(r, sc_r)
                    trs[r - 1] = emit_tr(pts[r - 1][0])
                    emit_pv(r - 2, trs[r - 2], pts[r - 2][1])
                trs[RT - 1] = emit_tr(pts[RT - 1][0])
                emit_pv(RT - 2, trs[RT - 2], pts[RT - 2][1])
                emit_pv(RT - 1, trs[RT - 1], pts[RT - 1][1])
                nc.sync.dma_start(out=rs[b], in_=rs_t)

    nc.finalize()
    _BUILD_CACHE[UPS] = nc
    return nc


def _bf16(x):
    return np.ascontiguousarray(x.astype(ml_dtypes.bfloat16))


def _roundup16(n):
    return max(P, ((n + 15) // 16) * 16)


def kernel(input, mask, Wq, bq, Wk, bk, Wv, bv):
    input = np.asarray(input, dtype=np.float32)
    mask = np.asarray(mask)
    scale = np.float32(1.0 / np.sqrt(H))

    # Fused scores: M = Wq^T Wk / sqrt(H); the bq row term feeds the
    # column bias c (folded into mkb), the bk row term feeds d.
    Wq = np.asarray(Wq, dtype=np.float32)
    Wk = np.asarray(Wk, dtype=np.float32)
    bq = np.asarray(bq, dtype=np.float32)
    bk = np.asarray(bk, dtype=np.float32)
    M = (Wq.T @ Wk) * scale
    w1 = (bq * scale) @ Wk               # column term: c[u] = XU[u] . w1
    bv = np.asarray(bv, dtype=np.float32)
    # The q-side row bias (bq term along queries) shifts every score in a row
    # equally, so it cancels in the softmax and is dropped entirely; bv is
    # added to att on the host since the weights sum to 1 per row.
    # Pre-tile weights: per-output-block, partition-major [blk, p, t, inner].
    # mTp holds M^T tiles (stationary for W2 = M @ XU^T).
    mTp = np.ascontiguousarray(
        _bf16(M.T).reshape(KT, P, KT, P).transpose(2, 1, 0, 3))
    wvp = np.ascontiguousarray(
        _bf16(np.asarray(Wv).T).reshape(KT, P, H // NFREE, NFREE).transpose(2, 1, 0, 3))

    # Permute each batch's token axis so unmasked tokens form a prefix: the
    # compact key/value block is then a slice of the (permuted) xT tile and
    # needs no separate transfer.  Queries are order-independent; outputs are
    # un-permuted below.
    m = np.asarray(mask[:, 0, 0, :])                     # [B, S]
    idxs = [np.nonzero(m[b] != 0)[0] for b in range(B)]
    ucounts = [len(ix) for ix in idxs]
    sparse = min(ucounts) > 0 and max(ucounts) < S
    if sparse:
        perms = [np.concatenate([idxs[b], np.nonzero(m[b] == 0)[0]]) for b in range(B)]
        # Assign batches to (core, slot) by descending unmasked count: slot 0
        # takes the 8 widest, slot 1 compiles against a narrower UP.
        order = np.argsort(np.asarray(ucounts), kind="stable")[::-1]
        asg = [[int(order[sl * NCORES + c]) for sl in range(B_LOC)]
               for c in range(NCORES)]                   # asg[core][slot] = batch
        UPS = [_roundup16(max(ucounts[asg[c][sl]] for c in range(NCORES)))
               for sl in range(B_LOC)]
    else:
        idxs = [np.arange(S) for _ in range(B)]
        ucounts = [S] * B
        perms = [np.arange(S) for _ in range(B)]
        asg = [[c * B_LOC + sl for sl in range(B_LOC)] for c in range(NCORES)]
        UPS = [S] * B_LOC
    UP0 = UPS[0]

    in_maps = []
    for c in range(NCORES):
        gbs = asg[c]
        xb = np.stack([input[gb][perms[gb]] for gb in gbs])  # [B_LOC, S, H]
        xTf = _bf16(xb.transpose(0, 2, 1))               # [B_LOC, H, S]
        mkb = np.zeros((B_LOC, P, UP0), dtype=ml_dtypes.bfloat16)
        for sl in range(B_LOC):
            gb = gbs[sl]
            UPb = UPS[sl]
            cvec = (xb[sl, :UPb].astype(np.float32) @ w1).astype(np.float32)
            mb = np.where(m[gb][perms[gb]][:UPb] == 0, np.float32(-1e9),
                          np.float32(0.0)) + cvec
            mkb[sl, :, :UPb] = mb.astype(ml_dtypes.bfloat16)[None, :]
        xT_t = np.ascontiguousarray(
            xTf.reshape(B_LOC, KT, P, S).transpose(0, 2, 1, 3))
        in_maps.append({
            "xT": xT_t,
            "mTp": mTp, "wvp": wvp, "mkb": mkb,
        })

    nc = build(UPS)
    res = run_bass_kernel_spmd(nc, in_maps, core_ids=list(range(NCORES)))
    att = np.empty((B, S, H), dtype=np.float32)
    attw = np.zeros((B, S, S), dtype=np.float32)
    for c in range(NCORES):
        att_c = res.results[c]["att"]                    # [B_LOC, S, H] bf16, permuted rows
        awc = res.results[c]["attwc"]                    # [B_LOC, S, UP0] bf16 raw exp
        rsc = res.results[c]["rs"]                       # [B_LOC, P, RT] f32 row sums
        for sl in range(B_LOC):
            gb = asg[c][sl]
            att[gb][perms[gb]] = att_c[sl].astype(np.float32) + bv
            rows = np.asarray(rsc[sl]).transpose(1, 0).reshape(S)   # per-query sums
            tmp = np.zeros((S, S), dtype=np.float32)
            tmp[:, idxs[gb]] = (awc[sl][:, :ucounts[gb]].astype(np.float32)
                                / rows[:, None])
            attw[gb][perms[gb]] = tmp
    return att, attw
